# revision 34
# baseline (speedup 1.0000x reference)
"""Trainium2 Bass kernel for nn_Attention_28338194219036.

GQA attention block (QKV proj + QK-RMSNorm + RoPE + causal SDPA + out proj)
for x:[2,2048,2048], 16 q-heads / 4 kv-heads, head_dim 128.

Distribution over 8 NeuronCores: 2-way data parallel on batch x 4-way tensor
parallel on heads (core c: batch c//4, TP rank c%4 -> q-heads 4r..4r+3,
kv-head r). Per 512-token chunk a 4-rank AllGather exchanges head-shards of
y^T; each rank then computes its 512 output channels of Wo for that chunk.

v2 design (vs baseline):
- All matmul operands bf16 (same PE rate as f32r, half the DMA/SBUF bytes).
  PSUM accumulation stays f32.
- Host pre-lays every tensor in its exact SBUF layout ([128, N] with
  channel-blocks as column groups), so each load is ONE contiguous DMA.
  The baseline's 284 small DMAs serialized on ~650ns/descriptor HWDGE time.
- V is projected directly into its natural [token, head-dim] layout
  (x as lhsT), eliminating the PE transposes.
- Emission interleaves projection chunks, attention waves, AllGathers and
  out-projection chunks so the in-order PE never waits on a collective:
  proj0 proj1 attn0 AG0 proj2 attn1 AG1 op0 proj3 attn2 AG2 op1
  attn3a AG3a op2 attn3b AG3b op3a op3b.  The final 512-token chunk is
  split in two so the last AllGather hides behind out-projection PE work.
- Attention waves walk key blocks in ascending order (diagonal blocks
  last) so a fresh chunk's attention does not wait on its own RMSNorm/RoPE
  chain; the softmax denominator accumulates on the PE via a ones-column
  matmul; no max-subtraction is needed since QK-RMSNorm bounds scores.
"""

import os
import sys

for _p in ("/opt/trn_rl_repo", "/root/.axon_site/_ro/trn_rl_repo"):
    if os.path.isdir(_p) and _p not in sys.path:
        sys.path.append(_p)

import numpy as np

B, T, C = 2, 2048, 2048
NH, NKV, HD = 16, 4, 128
TP = 4            # tensor-parallel group size
NCORES = 8
QH = NH // TP     # q-heads per core (4)
QD = QH * HD      # q channels per core (512)
TC = 4            # projection token chunks of 512
TCH = T // TC     # 512
CCH = C // 128    # 16 channel chunks
ROPE_BASE = 10000.0
SCALE = 1.0 / float(np.sqrt(HD))
EPS = float(np.finfo(np.float32).eps)
NEG = -1.0e9
REPEAT = 1
NO_COLLECTIVE = False

# attention sub-chunks: (qoff, qlen); last projection chunk split in two so the
# final AllGather+readback hides behind out-projection PE work
SUBS = [(0, 512), (512, 512), (1024, 512), (1536, 256), (1792, 256)]

_CACHE = {}


def _build_nc():
    import concourse.mybir as mybir
    import concourse.tile as tile
    from concourse import bacc

    F32 = mybir.dt.float32
    F32R = mybir.dt.float32r
    BF16 = mybir.dt.bfloat16
    AF = mybir.ActivationFunctionType

    nc = bacc.Bacc("TRN2", target_bir_lowering=False, debug=False, num_devices=NCORES)

    x_in = nc.dram_tensor("x_in", [128, TC * CCH * TCH], BF16, kind="ExternalInput")
    wq_in = nc.dram_tensor("wq_in", [128, CCH * QD], BF16, kind="ExternalInput")
    wk_in = nc.dram_tensor("wk_in", [128, CCH * HD], BF16, kind="ExternalInput")
    wv_in = nc.dram_tensor("wv_in", [128, CCH * HD], BF16, kind="ExternalInput")
    wo_in = nc.dram_tensor("wo_in", [128, CCH * QD], BF16, kind="ExternalInput")
    cc_in = nc.dram_tensor("cc_in", [128, T], F32, kind="ExternalInput")
    ss_in = nc.dram_tensor("ss_in", [128, T], F32, kind="ExternalInput")
    mask_in = nc.dram_tensor("mask_in", [128, 128], F32, kind="ExternalInput")
    outT = nc.dram_tensor("outT", [QD, T], F32, kind="ExternalOutput")

    with tile.TileContext(nc) as tc:
        for _rep in range(REPEAT):
            with (
                tc.tile_pool(name="drp", bufs=1, space="DRAM") as drp,
                tc.tile_pool(name="pw", bufs=1) as pw,
                tc.tile_pool(name="px", bufs=1) as px,
                tc.tile_pool(name="pat", bufs=1) as pat,
                tc.tile_pool(name="psp", bufs=1, space="PSUM") as psp,
            ):
                y_loc = [drp.tile([QD, ql], BF16, name=f"y_loc{i}") for i, (_, ql) in enumerate(SUBS)]
                y_all = [drp.tile([C, ql], BF16, name=f"y_all{i}") for i, (_, ql) in enumerate(SUBS)]

                # ---- persistent SBUF state ----
                ones_b = pw.tile([128, 1], BF16, name="ones_b")
                nc.any.memset(ones_b[:], 1.0)
                epst = pw.tile([1, 1], F32, name="epst")
                nc.any.memset(epst[:], EPS)

                wk_s = pw.tile([128, CCH * HD], BF16, name="wk_s")
                nc.sync.dma_start(wk_s[:, : CCH * HD // 2], wk_in[:, : CCH * HD // 2])
                wq_s = pw.tile([128, CCH * QD], BF16, name="wq_s")
                wv_s = pw.tile([128, CCH * HD], BF16, name="wv_s")
                wo_s = pw.tile([128, CCH * QD], BF16, name="wo_s")
                mask_tri = pw.tile([128, 128], F32, name="mask_tri")

                qhat = [pw.tile([128, T], BF16, name=f"qhat{h}") for h in range(QH)]
                khat = pw.tile([128, T], BF16, name="khat")
                vnat = pw.tile([128, T], BF16, name="vnat")

                def load_x(tci, first=False):
                    """One x chunk -> SBUF [128, CCH*TCH] bf16, split in 4 DMAs
                    so the first projection matmuls start early."""
                    x_t = px.tile([128, CCH * TCH], BF16, tag="x", bufs=2, name=f"x{tci}")
                    base = tci * CCH * TCH
                    step = 4 * TCH
                    for i in range(4):
                        nc.sync.dma_start(
                            x_t[:, i * step : (i + 1) * step],
                            x_in[:, base + i * step : base + (i + 1) * step],
                        )
                    return x_t

                def load_tabs(tci):
                    tsl = slice(tci * TCH, (tci + 1) * TCH)
                    cc_t = px.tile([128, TCH], F32, tag="cc", bufs=2, name=f"cc{tci}")
                    nc.sync.dma_start(cc_t[:], cc_in[:, tsl])
                    ss_t = px.tile([128, TCH], F32, tag="ss", bufs=2, name=f"ss{tci}")
                    nc.sync.dma_start(ss_t[:], ss_in[:, tsl])
                    return cc_t, ss_t

                x_tiles = {0: load_x(0, first=True)}
                # remaining big loads, in the order the PE will need them
                nc.sync.dma_start(wk_s[:, CCH * HD // 2 :], wk_in[:, CCH * HD // 2 :])
                tab_tiles = {0: load_tabs(0)}
                nc.sync.dma_start(wv_s[:], wv_in[:])
                nc.sync.dma_start(wq_s[:], wq_in[:])
                nc.sync.dma_start(mask_tri[:], mask_in[:])
                nc.sync.dma_start(wo_s[:], wo_in[:])

                def norm_rope(x_ps, dest_slice, cc_t, ss_t):
                    """RMSNorm + RoPE a [128(hd), 512(tok)] psum chunk into
                    dest_slice (bf16 sbuf). The psum tile is consumed by two
                    quick act-engine reads (Square + Copy) so its bank frees
                    long before the DVE chain drains."""
                    sq = px.tile([128, TCH], BF16, tag="sq", bufs=2, name="sq")
                    nc.scalar.activation(sq[:], x_ps[:], AF.Square)
                    xs = px.tile([128, TCH], F32, tag="xs", bufs=2, name="xs")
                    nc.scalar.activation(xs[:], x_ps[:], AF.Copy)
                    msq = psp.tile([1, TCH], F32, tag="sm", bufs=2, name="msq")
                    nc.tensor.matmul(msq[:], lhsT=ones_b[:], rhs=sq[:], start=True, stop=True)
                    srt = px.tile([1, TCH], F32, tag="srt", bufs=2, name="srt")
                    nc.scalar.activation(srt[:], msq[:], AF.Sqrt, bias=epst[:], scale=1.0 / HD)
                    rin = px.tile([1, TCH], F32, tag="rin", bufs=2, name="rin")
                    nc.vector.reciprocal(rin[:], srt[:])
                    rbc = px.tile([128, TCH], F32, tag="rbc", bufs=2, name="rbc")
                    nc.gpsimd.partition_broadcast(rbc[:], rin[:])
                    # RoPE: xhat = x*cc + swap64(x*ss_preswapped)
                    t1 = px.tile([128, TCH], F32, tag="t1", bufs=2, name="t1")
                    nc.vector.tensor_mul(t1[:], xs[:], ss_t[:])
                    t2 = px.tile([128, TCH], F32, tag="t2", bufs=2, name="t2")
                    nc.sync.dma_start(t2[0:64, :], t1[64:128, :])
                    nc.sync.dma_start(t2[64:128, :], t1[0:64, :])
                    u = px.tile([128, TCH], F32, tag="u", bufs=2, name="u")
                    nc.vector.tensor_mul(u[:], xs[:], cc_t[:])
                    nc.vector.tensor_add(u[:], u[:], t2[:])
                    nc.vector.tensor_mul(dest_slice, u[:], rbc[:])

                def proj(tci):
                    # prefetch next chunk's x + tables first
                    if tci + 1 < TC:
                        x_tiles[tci + 1] = load_x(tci + 1)
                        tab_tiles[tci + 1] = load_tabs(tci + 1)
                    x_t = x_tiles.pop(tci)
                    cc_t, ss_t = tab_tiles.pop(tci)
                    tsl = slice(tci * TCH, (tci + 1) * TCH)

                    # K projection
                    k_ps = psp.tile([128, TCH], F32, tag="G", bufs=4, name="k_ps")
                    for cci in range(CCH):
                        nc.tensor.matmul(
                            k_ps[:], lhsT=wk_s[:, cci * HD : (cci + 1) * HD],
                            rhs=x_t[:, cci * TCH : (cci + 1) * TCH],
                            start=(cci == 0), stop=(cci == CCH - 1),
                        )
                    norm_rope(k_ps, khat[:, tsl], cc_t, ss_t)

                    # V projection, directly in [token, hd] layout (x as lhsT)
                    v_ps = psp.tile([128, TCH], F32, tag="G", bufs=4, name="v_ps")
                    for jj in range(4):
                        for cci in range(CCH):
                            nc.tensor.matmul(
                                v_ps[:, jj * HD : (jj + 1) * HD],
                                lhsT=x_t[:, cci * TCH + jj * 128 : cci * TCH + (jj + 1) * 128],
                                rhs=wv_s[:, cci * HD : (cci + 1) * HD],
                                start=(cci == 0), stop=(cci == CCH - 1),
                            )
                    nc.any.tensor_copy(vnat[:, tsl], v_ps[:])

                    # Q projections
                    for h in range(QH):
                        q_ps = psp.tile([128, TCH], F32, tag="G", bufs=4, name="q_ps")
                        for cci in range(CCH):
                            nc.tensor.matmul(
                                q_ps[:], lhsT=wq_s[:, cci * QD + h * HD : cci * QD + (h + 1) * HD],
                                rhs=x_t[:, cci * TCH : (cci + 1) * TCH],
                                start=(cci == 0), stop=(cci == CCH - 1),
                            )
                        norm_rope(q_ps, qhat[h][:, tsl], cc_t, ss_t)

                def attn(si):
                    qoff, qlen = SUBS[si]
                    kb_tot = (qoff + qlen) // 128
                    dstart = qoff // 128  # first diagonal key block
                    yh = pat.tile([128, QH * TCH], BF16, tag="yh", bufs=2, name="yh")
                    LEAD = 3  # score blocks emitted ahead of their l/y pair
                    ps = {}

                    def s_exp(h, j):
                        off = max(0, j * 128 - qoff)
                        s_ps = psp.tile([128, TCH], F32, tag="G", bufs=4, name="s_ps")
                        nc.tensor.matmul(
                            s_ps[:, off:qlen],
                            lhsT=khat[:, j * 128 : (j + 1) * 128],
                            rhs=qhat[h][:, qoff + off : qoff + qlen],
                            start=True, stop=True,
                        )
                        if j >= dstart:
                            nc.vector.tensor_add(
                                s_ps[:, off : off + 128],
                                s_ps[:, off : off + 128],
                                mask_tri[:],
                            )
                        p = pat.tile([128, TCH], BF16, tag="p", bufs=8, name="p")
                        nc.scalar.activation(p[:, off:qlen], s_ps[:, off:qlen], AF.Exp, scale=SCALE)
                        ps[(h, j)] = p

                    # software-pipelined across head waves: the in-order PE
                    # runs the lookahead score matmul (possibly the next
                    # head's) while exp(j) is still on the act engine, so the
                    # l/y accumulators never head-of-line block the queue
                    blocks = [(h, j) for h in range(QH) for j in range(kb_tot)]
                    for h, j in blocks[:LEAD]:
                        s_exp(h, j)
                    y_ps = l_ps = None
                    for idx, (h, j) in enumerate(blocks):
                        if idx + LEAD < len(blocks):
                            s_exp(*blocks[idx + LEAD])
                        if j == 0:
                            y_ps = psp.tile([128, TCH], F32, tag="y", bufs=2, name="y_ps")
                            l_ps = psp.tile([1, TCH], F32, tag="sm", bufs=2, name="l_ps")
                        off = max(0, j * 128 - qoff)
                        p = ps.pop((h, j))
                        nc.tensor.matmul(
                            l_ps[:, off:qlen], lhsT=ones_b[:], rhs=p[:, off:qlen],
                            start=(j == 0), stop=(j == kb_tot - 1),
                        )
                        nc.tensor.matmul(
                            y_ps[:, off:qlen], lhsT=vnat[:, j * 128 : (j + 1) * 128],
                            rhs=p[:, off:qlen],
                            start=(j == 0), stop=(j == kb_tot - 1),
                        )
                        if j == kb_tot - 1:
                            rl = pat.tile([1, TCH], F32, tag="rl", bufs=2, name="rl")
                            nc.vector.reciprocal(rl[:, :qlen], l_ps[:, :qlen])
                            rb = pat.tile([128, TCH], F32, tag="rb", bufs=2, name="rb")
                            nc.gpsimd.partition_broadcast(rb[:, :qlen], rl[:, :qlen])
                            nc.vector.tensor_mul(
                                yh[:, h * qlen : (h + 1) * qlen], y_ps[:, :qlen], rb[:, :qlen]
                            )
                    # ship the 4 head-shards to DRAM in one DMA
                    src = yh[:, : QH * qlen].rearrange("p (h t) -> p h t", h=QH)
                    dst = y_loc[si][:].rearrange("(h p) t -> p h t", p=128)
                    nc.sync.dma_start(dst, src)

                def allgather(si):
                    if NO_COLLECTIVE:
                        for q in range(TP):
                            nc.sync.dma_start(
                                y_all[si][q * QD : (q + 1) * QD, :], y_loc[si][:]
                            )
                    else:
                        import concourse.mybir as mybir

                        nc.gpsimd.collective_compute(
                            "AllGather",
                            mybir.AluOpType.bypass,
                            replica_groups=[[0, 1, 2, 3], [4, 5, 6, 7]],
                            ins=[y_loc[si][:]],
                            outs=[y_all[si][:]],
                        )

                yb_tiles = {}

                def oread(si):
                    # issue the gathered-y readback as soon as the AllGather is
                    # ordered, so it never queues behind later DMA traffic
                    qoff, qlen = SUBS[si]
                    yb = pat.tile([128, CCH * TCH], BF16, tag="yb", bufs=2, name="yb")
                    src = y_all[si][:].rearrange("(cci p) t -> p cci t", p=128)
                    dst = yb[:, : CCH * qlen].rearrange("p (cci t) -> p cci t", t=qlen)
                    nc.sync.dma_start(dst, src)
                    yb_tiles[si] = yb

                def outproj(si):
                    qoff, qlen = SUBS[si]
                    yb = yb_tiles.pop(si)
                    ob = pat.tile([128, 4 * TCH], F32, tag="ob", bufs=1, name="ob")
                    for jq in range(4):
                        o_ps = psp.tile([128, TCH], F32, tag="G", bufs=4, name="o_ps")
                        for cci in range(CCH):
                            nc.tensor.matmul(
                                o_ps[:, :qlen],
                                lhsT=wo_s[:, cci * QD + jq * 128 : cci * QD + (jq + 1) * 128],
                                rhs=yb[:, cci * qlen : (cci + 1) * qlen],
                                start=(cci == 0), stop=(cci == CCH - 1),
                            )
                        nc.vector.tensor_copy(ob[:, jq * qlen : (jq + 1) * qlen], o_ps[:, :qlen])
                    src = ob[:, : 4 * qlen].rearrange("p (jq t) -> p jq t", jq=4)
                    dst = outT[:, qoff : qoff + qlen].rearrange("(jq p) t -> p jq t", p=128)
                    nc.sync.dma_start(dst, src)

                # ---- emission schedule ----
                proj(0)
                proj(1)
                attn(0); allgather(0); oread(0)
                proj(2)
                attn(1); allgather(1); oread(1); outproj(0)
                proj(3)
                attn(2); allgather(2); oread(2); outproj(1)
                attn(3); allgather(3); oread(3); outproj(2)
                attn(4); allgather(4); oread(4); outproj(3)
                outproj(4)

    nc.compile()
    return nc


def _get_nc():
    if "nc" not in _CACHE:
        _CACHE["nc"] = _build_nc()
    return _CACHE["nc"]


def _lay(wT):
    """[C, M] (already transposed weight) -> [128, (C/128)*M] with channel
    blocks as column groups: out[p, cci*M + j] = wT[cci*128 + p, j]."""
    Cd, M = wT.shape
    return np.ascontiguousarray(
        wT.reshape(Cd // 128, 128, M).transpose(1, 0, 2).reshape(128, -1)
    )


def _host_constants():
    if "consts" in _CACHE:
        return _CACHE["consts"]
    inv_freq = 1.0 / (ROPE_BASE ** (np.arange(0, HD, 2, dtype=np.float64) / HD))
    freqs = np.outer(np.arange(T, dtype=np.float64), inv_freq)  # [T, 64]
    cos = np.cos(freqs).astype(np.float32).T  # [64, T]
    sin = np.sin(freqs).astype(np.float32).T
    ccT = np.ascontiguousarray(np.concatenate([cos, cos], axis=0))   # [128, T]
    # the kernel computes swap(x*ss) (swap applied AFTER the multiply), so the
    # sin table is pre-swapped: swap(x)*[+sin;-sin] == swap(x*[-sin;+sin])
    ssT = np.ascontiguousarray(np.concatenate([-sin, sin], axis=0))  # [128, T]
    ii = np.arange(128, dtype=np.int64)[:, None]
    cc = np.arange(128, dtype=np.int64)[None, :]
    masks = np.where(cc >= ii, 0.0, NEG).astype(np.float32)
    perm = np.zeros((128, 128), dtype=np.float32)
    perm[(np.arange(128) + 64) % 128, np.arange(128)] = 1.0
    _CACHE["consts"] = (ccT, ssT, masks, perm)
    return _CACHE["consts"]


def _in_maps(x, Wq, Wk, Wv, Wo):
    import ml_dtypes

    BF = ml_dtypes.bfloat16
    ccT, ssT, masks, perm = _host_constants()
    maps = []
    for c in range(NCORES):
        b, r = divmod(c, TP)
        xT = x[b].T.astype(BF)  # [C, T]
        x_l = np.concatenate(
            [_lay(xT[:, t * TCH : (t + 1) * TCH]) for t in range(TC)], axis=1
        )
        maps.append(
            {
                "x_in": np.ascontiguousarray(x_l),
                "wq_in": _lay(Wq[r * QD : (r + 1) * QD, :].T.astype(BF)),
                "wk_in": _lay(Wk[r * HD : (r + 1) * HD, :].T.astype(BF)),
                "wv_in": _lay(Wv[r * HD : (r + 1) * HD, :].T.astype(BF)),
                "wo_in": _lay(Wo[r * QD : (r + 1) * QD, :].T.astype(BF)),
                "cc_in": ccT,
                "ss_in": ssT,
                "mask_in": masks,
            }
        )
    return maps


def _assemble(results):
    out = np.empty((B, T, C), dtype=np.float32)
    for c in range(NCORES):
        b, r = divmod(c, TP)
        out[b, :, r * QD : (r + 1) * QD] = results[c]["outT"].T
    return out


def kernel(x, Wq, Wk, Wv, Wo):
    from concourse.bass_utils import run_bass_kernel_spmd

    nc = _get_nc()
    maps = _in_maps(np.asarray(x), np.asarray(Wq), np.asarray(Wk), np.asarray(Wv), np.asarray(Wo))
    res = run_bass_kernel_spmd(nc, maps, list(range(NCORES)))
    return _assemble(res.results)
